# revision 9
# baseline (speedup 1.0000x reference)
"""Trainium2 Bass kernel for nn_Recon_block (block-sparse attention recon).

Math (per 48x48 block, c=31 channels, N=2304 tokens):
  x:   (c, N) block pixels
  xg = w3 @ x                      (1x1 conv -> value tensor)
  S  = x^T x                       (N, N) symmetric score matrix
  P  = exp(S / sqrt(c))
  ctx[m, :] = (P[m, :] @ xg^T) / sum_n P[m, n] * (48/78)
  out = relu(ctx) viewed as (c, 48, 48) raw buffer reinterpretation

Device computes ctxT = [xg; ones] @ P  -> (32, N) per block (row 31 = softmax
denominator); host does the divide / scale / relu / relayout (tiny).

Key device tricks:
  * S is symmetric (q = k = x): only the lower block-triangle of P is
    computed (matmul + ACT exp); upper-triangle tiles are produced by
    SBUF->SBUF DMA-transpose of the exp'd mirror tile (bf16, X-bar), which
    moves ~40% of the softmax-exp work off the Scalar engine.
  * Score matmuls have K=31: two concurrent matmuls packed into the PE
    array via rotating 32-row quadrants of a 4-replica x layout.
  * PV matmuls have M=32: four concurrent matmuls packed via 32-column PSUM
    strips, summed afterwards on the Vector engine.

Sharding: 36 independent blocks data-parallel over 8 cores (5 slots per
core; cores 4..7 have one duplicated slot whose output is discarded).
"""

import numpy as np
import ml_dtypes
from contextlib import ExitStack

import concourse.bass as bass
import concourse.tile as tile
from concourse import bacc, mybir
from concourse import bass_utils

BLK = 48
C = 31
N = BLK * BLK  # 2304
NCORES = 8
SLOTS = 5
NT = N // 128  # 18 n-tiles
SCALE = 1.0 / float(np.sqrt(C))
CTX_SCALE = BLK / (BLK + C - 1.0)  # 48/78

GROUPS = [(0, 512), (512, 512), (1024, 512), (1536, 512), (2048, 256)]
AR = 2  # a-tiles per score-matmul round (PSUM budget bound)
BF16 = mybir.dt.bfloat16
F32 = mybir.dt.float32

_BUILT = {}


def _build(slots=SLOTS, n_cores=NCORES, repeat=1, mirrors=True):
    key = (slots, n_cores, repeat, mirrors)
    if key in _BUILT:
        return _BUILT[key]
    nc = bacc.Bacc("TRN2", target_bir_lowering=False, debug=False,
                   num_devices=n_cores)
    xb = nc.dram_tensor("xb", [slots, C, N], BF16, kind="ExternalInput").ap()
    w3t = nc.dram_tensor("w3t", [C, C], BF16, kind="ExternalInput").ap()
    out = nc.dram_tensor("out", [slots, C + 1, N], F32,
                         kind="ExternalOutput").ap()

    with tile.TileContext(nc) as tc, ExitStack() as ctx:
        const_pool = ctx.enter_context(tc.tile_pool(name="const", bufs=1))
        xpool = ctx.enter_context(tc.tile_pool(name="xpool", bufs=2))
        pk_pool = ctx.enter_context(tc.tile_pool(name="pk", bufs=1))
        xg_pool = ctx.enter_context(tc.tile_pool(name="xg", bufs=2))
        ctx_sb_pool = ctx.enter_context(tc.tile_pool(name="ctxsb", bufs=2))
        strip_pool = ctx.enter_context(tc.tile_pool(name="strips", bufs=2))
        psum_s = ctx.enter_context(
            tc.tile_pool(name="psum_s", bufs=2, space="PSUM"))
        psum_xg = ctx.enter_context(
            tc.tile_pool(name="psum_xg", bufs=1, space="PSUM"))
        psum_ctx = ctx.enter_context(
            tc.tile_pool(name="psum_ctx", bufs=1, space="PSUM"))
        psum_t = ctx.enter_context(
            tc.tile_pool(name="psum_t", bufs=2, space="PSUM"))

        w3t_sb = const_pool.tile([C, C], BF16)
        nc.sync.dma_start(w3t_sb[:], w3t[:])
        ident = const_pool.tile([128, 128], BF16)
        from concourse.masks import make_identity
        make_identity(nc, ident[:])

        for s in [s for _ in range(repeat) for s in range(slots)]:
            # x, replicated into the four 32-partition quadrants
            x4 = xpool.tile([128, N], BF16)
            for r in range(3):  # AP base partition must be in {0, 32, 64}
                nc.sync.dma_start(x4[32 * r:32 * r + C, :], xb[s])

            # xgT1[n, a, :31] = (w3 @ x)^T tile a; [..., 31] = 1.0
            xgT1 = xg_pool.tile([128, NT, 32], BF16)
            nc.vector.memset(xgT1[:], 1.0)
            for a0 in range(0, NT, 4):
                cnt = min(4, NT - a0)
                pxg = psum_xg.tile([128, 4, C], F32)
                for t in range(cnt):
                    a = a0 + t
                    nc.tensor.matmul(
                        pxg[:, t, :],
                        lhsT=x4[0:C, a * 128:(a + 1) * 128],
                        rhs=w3t_sb[:],
                        start=True, stop=True)
                nc.vector.tensor_copy(xgT1[:, a0:a0 + cnt, 0:C],
                                      pxg[:, 0:cnt, :])

            pk = pk_pool.tile([128, NT, N], BF16)  # P^T, [n-in-tile, a, m]
            ctxT = ctx_sb_pool.tile([C + 1, N], F32)

            def emit_st(gi):
                """Score matmuls + exp for group gi (direct tiles only)."""
                g0, W = GROUPS[gi]
                a_start = (g0 // 128) if mirrors else 0
                for a0 in range(a_start, NT, AR):
                    cnt = min(AR, NT - a0)
                    ps = psum_s.tile([128, AR, 512], F32, name="ps")
                    for t in range(cnt):
                        a = a0 + t
                        r = a % 3  # quadrant -> PE row group
                        nc.tensor.matmul(
                            ps[:, t, 0:W],
                            lhsT=x4[32 * r:32 * r + C,
                                    a * 128:(a + 1) * 128],
                            rhs=x4[32 * r:32 * r + C, g0:g0 + W],
                            start=True, stop=True)
                    nc.scalar.activation(
                        pk[:, a0:a0 + cnt, g0:g0 + W], ps[:, 0:cnt, 0:W],
                        mybir.ActivationFunctionType.Exp, scale=SCALE)

            def emit_mirrors(gi):
                # Upper-triangle tiles of P^T by symmetry: PE-transpose the
                # exp'd lower-triangle mirror tile (bf16 via PSUM), evacuate
                # four tiles per DVE copy (2x bf16 mode).
                g0, W = GROUPS[gi]
                k = g0 // 128
                for b in range(g0 // 128, (g0 + W) // 128):
                    for a0 in range(0, k, 4):
                        cnt = min(4, k - a0)
                        pt = psum_t.tile([128, 4, 128], BF16, name="pt")
                        for t in range(cnt):
                            a = a0 + t
                            nc.tensor.transpose(
                                pt[:, t, :],
                                pk[:, b, a * 128:(a + 1) * 128],
                                ident[:])
                        nc.vector.tensor_copy(
                            pk[:, a0:a0 + cnt, b * 128:(b + 1) * 128],
                            pt[:, 0:cnt, :])

            def emit_pv(gi):
                g0, W = GROUPS[gi]
                pctx = psum_ctx.tile([128, 512], F32, name="pctx")
                for a in range(NT):
                    t = a % 3  # PE column group / PSUM strip
                    nc.tensor.matmul(
                        pctx[32 * t:32 * t + 32, 0:W],
                        lhsT=xgT1[:, a, :],
                        rhs=pk[:, a, g0:g0 + W],
                        start=(a < 3), stop=(a >= NT - 3),
                        skip_group_check=True)
                # DVE can read only one PSUM operand per instruction
                t0 = strip_pool.tile([32, 512], F32, name="t0")
                nc.vector.tensor_copy(t0[:, 0:W], pctx[0:32, 0:W])
                t01 = strip_pool.tile([32, 512], F32, name="t01")
                nc.vector.tensor_add(t01[:, 0:W], t0[:, 0:W],
                                     pctx[32:64, 0:W])
                nc.vector.tensor_add(ctxT[:, g0:g0 + W], t01[:, 0:W],
                                     pctx[64:96, 0:W])

            # software pipeline at group level: ST(g+1) overlaps PV(g)
            for gi in range(len(GROUPS)):
                if mirrors:
                    emit_mirrors(gi)
                emit_st(gi)
                if gi > 0:
                    emit_pv(gi - 1)
            emit_pv(len(GROUPS) - 1)

            nc.sync.dma_start(out[s], ctxT[:])

    nc.compile()
    _BUILT[key] = nc
    return nc


def _conv2d_np(x, w, pad):
    """x: (ci, h, w) f32; w: (co, ci, kh, kw); zero padding `pad`."""
    ci, h, wd = x.shape
    co, _, kh, kw = w.shape
    xp = np.zeros((ci, h + 2 * pad, wd + 2 * pad), np.float32)
    xp[:, pad:pad + h, pad:pad + wd] = x
    out = np.zeros((co, h, wd), np.float32)
    for dy in range(kh):
        for dx in range(kw):
            out += np.einsum('oi,iyx->oyx', w[:, :, dy, dx],
                             xp[:, dy:dy + h, dx:dx + wd],
                             optimize=True)
    return out


def kernel(xt, w1, w2, w3, wz1):
    xt = np.asarray(xt)
    w1 = np.asarray(w1)
    w2 = np.asarray(w2)
    w3 = np.asarray(w3)
    wz1 = np.asarray(wz1)

    b, c, h_inp, w_inp = xt.shape
    assert (b, c, h_inp, w_inp) == (1, C, 256, 256)
    pad_h = (-h_inp) % BLK
    pad_w = (-w_inp) % BLK
    xpad = np.pad(xt, ((0, 0), (0, 0), (0, pad_h), (0, pad_w)), mode='reflect')
    H, W = h_inp + pad_h, w_inp + pad_w
    nh, nw = H // BLK, W // BLK
    nblk = nh * nw  # 36
    xb_all = xpad.reshape(c, nh, BLK, nw, BLK).transpose(1, 3, 0, 2, 4)
    xb_all = np.ascontiguousarray(xb_all).reshape(nblk, c, N)

    counts = [5, 5, 5, 5, 4, 4, 4, 4]
    starts = np.concatenate([[0], np.cumsum(counts)])[:NCORES]
    xb_bf16 = xb_all.astype(ml_dtypes.bfloat16)
    w3t = np.ascontiguousarray(w3[:, :, 0, 0].T).astype(ml_dtypes.bfloat16)

    in_maps = []
    for ci_ in range(NCORES):
        blocks = [starts[ci_] + min(s, counts[ci_] - 1) for s in range(SLOTS)]
        in_maps.append({"xb": np.ascontiguousarray(xb_bf16[blocks]),
                        "w3t": w3t})

    nc = _build()
    res = bass_utils.run_bass_kernel_spmd(
        nc, in_maps, core_ids=list(range(NCORES)))

    # Host-side epilogue: normalize, scale, relu, relayout.
    zpad = np.zeros((c, H, W), np.float32)
    use_conv = float(wz1.reshape(-1)[0]) != 0.0
    wz = np.float32(wz1.reshape(-1)[0])
    for ci_ in range(NCORES):
        o = res.results[ci_]["out"]  # [SLOTS, 32, N] f32
        for s in range(counts[ci_]):
            blk = starts[ci_] + s
            ctxT = o[s]
            denom = ctxT[C]
            ctxm = ctxT[:C] * (CTX_SCALE / denom)  # (c, N)
            zblk = np.maximum(ctxm.T, 0.0).reshape(c, BLK, BLK)
            if use_conv:
                xblk = xb_all[blk].reshape(c, BLK, BLK).astype(np.float32)
                h1 = np.maximum(_conv2d_np(xblk, w1, 1), 0.0)
                z1 = xblk + _conv2d_np(h1, w2, 1)
                zblk = wz * z1 + (1.0 - wz) * zblk
            i, j = blk // nw, blk % nw
            zpad[:, i * BLK:(i + 1) * BLK, j * BLK:(j + 1) * BLK] = zblk

    return zpad[None, :, :h_inp, :w_inp].astype(np.float32)


# revision 15
# speedup vs baseline: 1.0572x; 1.0572x over previous
"""Trainium2 Bass kernel for nn_Recon_block (block-sparse attention recon).

Math (per 48x48 block, c=31 channels, N=2304 tokens):
  x:   (c, N) block pixels
  xg = w3 @ x                      (1x1 conv -> value tensor)
  S  = x^T x                       (N, N) symmetric score matrix
  P  = exp(S / sqrt(c))
  ctx[m, :] = (P[m, :] @ xg^T) / sum_n P[m, n] * (48/78)
  out = relu(ctx) viewed as (c, 48, 48) raw buffer reinterpretation

Device computes ctxT = [xg; ones] @ P  -> (32, N) per block (row 31 = softmax
denominator); host does the divide / scale / relu / relayout (tiny).

Key device tricks:
  * S is symmetric (q = k = x): only the lower block-triangle of P is
    computed (matmul + ACT exp); upper-triangle tiles are produced by
    SBUF->SBUF DMA-transpose of the exp'd mirror tile (bf16, X-bar), which
    moves ~40% of the softmax-exp work off the Scalar engine.
  * Score matmuls have K=31: two concurrent matmuls packed into the PE
    array via rotating 32-row quadrants of a 4-replica x layout.
  * PV matmuls have M=32: four concurrent matmuls packed via 32-column PSUM
    strips, summed afterwards on the Vector engine.

Sharding: 36 independent blocks data-parallel over 8 cores (5 slots per
core; cores 4..7 have one duplicated slot whose output is discarded).
"""

import numpy as np
import ml_dtypes
from contextlib import ExitStack

import concourse.bass as bass
import concourse.tile as tile
from concourse import bacc, mybir
from concourse import bass_utils

BLK = 48
C = 31
N = BLK * BLK  # 2304
NCORES = 8
SLOTS = 5
NT = N // 128  # 18 n-tiles
SCALE = 1.0 / float(np.sqrt(C))
CTX_SCALE = BLK / (BLK + C - 1.0)  # 48/78

GROUPS = [(0, 512), (512, 512), (1024, 512), (1536, 512), (2048, 256)]
# last slot of each core processes only the first half of a (possibly
# token-rotated) block: 36 blocks = 8 cores x (4 full + 1 half)
GROUPS_HALF = [(0, 512), (512, 512), (1024, 128)]
HALF = N // 2  # 1152
AR = 2  # a-tiles per score-matmul round (PSUM budget bound)
BF16 = mybir.dt.bfloat16
F32 = mybir.dt.float32

_BUILT = {}


def _build(slots=SLOTS, n_cores=NCORES, repeat=1, mirrors=True):
    key = (slots, n_cores, repeat, mirrors)
    if key in _BUILT:
        return _BUILT[key]
    nc = bacc.Bacc("TRN2", target_bir_lowering=False, debug=False,
                   num_devices=n_cores)
    xb = nc.dram_tensor("xb", [slots, C, N], BF16, kind="ExternalInput").ap()
    w3t = nc.dram_tensor("w3t", [C, C], BF16, kind="ExternalInput").ap()
    out = nc.dram_tensor("out", [slots, C + 1, N], F32,
                         kind="ExternalOutput").ap()

    with tile.TileContext(nc) as tc, ExitStack() as ctx:
        const_pool = ctx.enter_context(tc.tile_pool(name="const", bufs=1))
        xpool = ctx.enter_context(tc.tile_pool(name="xpool", bufs=2))
        pk_pool = ctx.enter_context(tc.tile_pool(name="pk", bufs=1))
        xg_pool = ctx.enter_context(tc.tile_pool(name="xg", bufs=2))
        ctx_sb_pool = ctx.enter_context(tc.tile_pool(name="ctxsb", bufs=2))
        strip_pool = ctx.enter_context(tc.tile_pool(name="strips", bufs=2))
        psum_s = ctx.enter_context(
            tc.tile_pool(name="psum_s", bufs=2, space="PSUM"))
        psum_ctx = ctx.enter_context(
            tc.tile_pool(name="psum_ctx", bufs=2, space="PSUM"))
        # xg staging and transpose staging share slots (disjoint in time)
        psum_t = ctx.enter_context(
            tc.tile_pool(name="psum_t", bufs=2, space="PSUM"))
        psum_xg = psum_t

        w3t_sb = const_pool.tile([C, C], BF16)
        nc.sync.dma_start(w3t_sb[:], w3t[:])
        ident = const_pool.tile([128, 128], BF16)
        from concourse.masks import make_identity
        make_identity(nc, ident[:])

        half_last = (slots == SLOTS)
        for s in [s for _ in range(repeat) for s in range(slots)]:
            groups = GROUPS_HALF if (half_last and s == slots - 1) else GROUPS
            # x, replicated into the four 32-partition quadrants
            x4 = xpool.tile([128, N], BF16)
            for r in range(3):  # AP base partition must be in {0, 32, 64}
                nc.sync.dma_start(x4[32 * r:32 * r + C, :], xb[s])

            # xgT1[n, a, :31] = (w3 @ x)^T tile a; [..., 31] = 1.0
            xgT1 = xg_pool.tile([128, NT, 32], BF16)
            nc.vector.memset(xgT1[:], 1.0)
            for a0 in range(0, NT, 4):
                cnt = min(4, NT - a0)
                pxg = psum_xg.tile([128, 4, C], F32, tag="pt")
                for t in range(cnt):
                    a = a0 + t
                    nc.tensor.matmul(
                        pxg[:, t, :],
                        lhsT=x4[0:C, a * 128:(a + 1) * 128],
                        rhs=w3t_sb[:],
                        start=True, stop=True)
                nc.vector.tensor_copy(xgT1[:, a0:a0 + cnt, 0:C],
                                      pxg[:, 0:cnt, :])

            pk = pk_pool.tile([128, NT, N], BF16)  # P^T, [n-in-tile, a, m]
            ctxT = ctx_sb_pool.tile([C + 1, N], F32)

            def emit_st(gi):
                """Score matmuls + exp for group gi (direct tiles only)."""
                g0, W = groups[gi]
                a_start = (g0 // 128) if mirrors else 0
                for a0 in range(a_start, NT, AR):
                    cnt = min(AR, NT - a0)
                    ps = psum_s.tile([128, AR, 512], F32, name="ps")
                    for t in range(cnt):
                        a = a0 + t
                        r = a % 3  # quadrant -> PE row group
                        nc.tensor.matmul(
                            ps[:, t, 0:W],
                            lhsT=x4[32 * r:32 * r + C,
                                    a * 128:(a + 1) * 128],
                            rhs=x4[32 * r:32 * r + C, g0:g0 + W],
                            start=True, stop=True)
                    nc.scalar.activation(
                        pk[:, a0:a0 + cnt, g0:g0 + W], ps[:, 0:cnt, 0:W],
                        mybir.ActivationFunctionType.Exp, scale=SCALE)

            def emit_mirrors(gi):
                # Upper-triangle tiles of P^T by symmetry: PE-transpose the
                # exp'd lower-triangle mirror tile (bf16 via PSUM), evacuate
                # four tiles per DVE copy (2x bf16 mode).
                g0, W = groups[gi]
                k = g0 // 128
                for b in range(g0 // 128, (g0 + W) // 128):
                    for a0 in range(0, k, 4):
                        cnt = min(4, k - a0)
                        pt = psum_t.tile([128, 4, 128], BF16, name="pt")
                        for t in range(cnt):
                            a = a0 + t
                            nc.tensor.transpose(
                                pt[:, t, :],
                                pk[:, b, a * 128:(a + 1) * 128],
                                ident[:])
                        nc.vector.tensor_copy(
                            pk[:, a0:a0 + cnt, b * 128:(b + 1) * 128],
                            pt[:, 0:cnt, :])

            def emit_pv(gi):
                g0, W = groups[gi]
                pctx = psum_ctx.tile([128, 512], F32, name="pctx")
                for a in range(NT):
                    t = a % 3  # PE column group / PSUM strip
                    nc.tensor.matmul(
                        pctx[32 * t:32 * t + 32, 0:W],
                        lhsT=xgT1[:, a, :],
                        rhs=pk[:, a, g0:g0 + W],
                        start=(a < 3), stop=(a >= NT - 3),
                        skip_group_check=True)
                # DVE can read only one PSUM operand per instruction
                t0 = strip_pool.tile([32, 512], F32, name="t0")
                nc.vector.tensor_copy(t0[:, 0:W], pctx[0:32, 0:W])
                t01 = strip_pool.tile([32, 512], F32, name="t01")
                nc.vector.tensor_add(t01[:, 0:W], t0[:, 0:W],
                                     pctx[32:64, 0:W])
                nc.vector.tensor_add(ctxT[:, g0:g0 + W], t01[:, 0:W],
                                     pctx[64:96, 0:W])

            # software pipeline at group level: ST(g+1) overlaps PV(g)
            for gi in range(len(groups)):
                if mirrors:
                    emit_mirrors(gi)
                emit_st(gi)
                if gi > 0:
                    emit_pv(gi - 1)
            emit_pv(len(groups) - 1)

            mtot = sum(w for _, w in groups)
            nc.sync.dma_start(out[s][:, 0:mtot], ctxT[:, 0:mtot])

    nc.compile()
    _BUILT[key] = nc
    return nc


def make_in_maps(xb_all, w3):
    """Per-core input maps: 4 full blocks + 1 half block per core.

    Blocks 0..31 -> core c slots 0..3.  Blocks 32+k are split: core 2k's
    slot 4 sees the block as-is (computes tokens [0,1152)); core 2k+1's
    slot 4 sees it rotated by 1152 tokens (computes the other half --
    attention commutes with any joint token permutation)."""
    xb_bf16 = xb_all.astype(ml_dtypes.bfloat16)
    w3t = np.ascontiguousarray(w3[:, :, 0, 0].T).astype(ml_dtypes.bfloat16)
    in_maps = []
    for ci_ in range(NCORES):
        sl = [xb_bf16[4 * ci_ + s] for s in range(4)]
        xh = xb_bf16[32 + ci_ // 2]
        if ci_ % 2 == 1:
            xh = np.concatenate([xh[:, HALF:], xh[:, :HALF]], axis=1)
        sl.append(xh)
        in_maps.append({"xb": np.ascontiguousarray(np.stack(sl)),
                        "w3t": w3t})
    return in_maps


def _conv2d_np(x, w, pad):
    """x: (ci, h, w) f32; w: (co, ci, kh, kw); zero padding `pad`."""
    ci, h, wd = x.shape
    co, _, kh, kw = w.shape
    xp = np.zeros((ci, h + 2 * pad, wd + 2 * pad), np.float32)
    xp[:, pad:pad + h, pad:pad + wd] = x
    out = np.zeros((co, h, wd), np.float32)
    for dy in range(kh):
        for dx in range(kw):
            out += np.einsum('oi,iyx->oyx', w[:, :, dy, dx],
                             xp[:, dy:dy + h, dx:dx + wd],
                             optimize=True)
    return out


def kernel(xt, w1, w2, w3, wz1):
    xt = np.asarray(xt)
    w1 = np.asarray(w1)
    w2 = np.asarray(w2)
    w3 = np.asarray(w3)
    wz1 = np.asarray(wz1)

    b, c, h_inp, w_inp = xt.shape
    assert (b, c, h_inp, w_inp) == (1, C, 256, 256)
    pad_h = (-h_inp) % BLK
    pad_w = (-w_inp) % BLK
    xpad = np.pad(xt, ((0, 0), (0, 0), (0, pad_h), (0, pad_w)), mode='reflect')
    H, W = h_inp + pad_h, w_inp + pad_w
    nh, nw = H // BLK, W // BLK
    nblk = nh * nw  # 36
    xb_all = xpad.reshape(c, nh, BLK, nw, BLK).transpose(1, 3, 0, 2, 4)
    xb_all = np.ascontiguousarray(xb_all).reshape(nblk, c, N)

    in_maps = make_in_maps(xb_all, w3)

    nc = _build()
    res = bass_utils.run_bass_kernel_spmd(
        nc, in_maps, core_ids=list(range(NCORES)))

    # Host-side epilogue: normalize, scale, relu, relayout.
    ctxm_all = {}
    half_parts = {}
    for ci_ in range(NCORES):
        o = res.results[ci_]["out"]  # [SLOTS, 32, N] f32
        for s in range(4):
            ctxT = o[s]
            ctxm_all[4 * ci_ + s] = ctxT[:C] * (CTX_SCALE / ctxT[C])
        ctxTh = o[4][:, :HALF]
        half_parts.setdefault(32 + ci_ // 2, {})[ci_ % 2] = (
            ctxTh[:C] * (CTX_SCALE / ctxTh[C]))
    for blk, parts in half_parts.items():
        ctxm_all[blk] = np.concatenate([parts[0], parts[1]], axis=1)

    zpad = np.zeros((c, H, W), np.float32)
    use_conv = float(wz1.reshape(-1)[0]) != 0.0
    wz = np.float32(wz1.reshape(-1)[0])
    for blk in range(nblk):
        zblk = np.maximum(ctxm_all[blk].T, 0.0).reshape(c, BLK, BLK)
        if use_conv:
            xblk = xb_all[blk].reshape(c, BLK, BLK).astype(np.float32)
            h1 = np.maximum(_conv2d_np(xblk, w1, 1), 0.0)
            z1 = xblk + _conv2d_np(h1, w2, 1)
            zblk = wz * z1 + (1.0 - wz) * zblk
        i, j = blk // nw, blk % nw
        zpad[:, i * BLK:(i + 1) * BLK, j * BLK:(j + 1) * BLK] = zblk

    return zpad[None, :, :h_inp, :w_inp].astype(np.float32)


# revision 21
# speedup vs baseline: 3851.7464x; 3643.2859x over previous
"""Trainium2 Bass kernel for nn_Recon_block (block-sparse attention recon).

Math (per 48x48 block, c=31 channels, N=2304 tokens):
  x:   (c, N) block pixels
  xg = w3 @ x                      (1x1 conv -> value tensor)
  S  = x^T x                       (N, N) symmetric score matrix
  P  = exp(S / sqrt(c))
  ctx[m, :] = (P[m, :] @ xg^T) / sum_n P[m, n] * (48/78)
  out = relu(ctx) viewed as (c, 48, 48) raw buffer reinterpretation

Device computes ctxT = [xg; ones] @ P  -> (32, N) per block (row 31 = softmax
denominator); host does the divide / scale / relu / relayout (tiny).

Key device tricks:
  * S is symmetric (q = k = x): only the lower block-triangle of P is
    exp'd on the Scalar engine; upper-triangle tiles are PE-transposed
    copies of the exp'd mirror tile (bf16 via PSUM), evacuated by the
    Vector engine in its 2x bf16 mode. This moves ~40% of the softmax-exp
    wall off the Scalar engine, which is otherwise the bottleneck.
  * Score matmuls have K=31: concurrent matmuls are packed into the PE
    array via rotating 32-row quadrants of a replicated x layout
    (tile_size (32,128), positions inferred from partition bases).
  * PV matmuls have M=32: three concurrent matmuls packed via 32-row PSUM
    strips (tile_size (128,32)), strip-summed on the Vector engine. The
    value matrix carries an extra all-ones column producing the softmax
    denominator for free.
  * Group-level software pipeline: mirrors(g) | scores+exp(g) | PV(g-1).

Sharding: 36 independent blocks over 8 cores as 4 full blocks + 1 half
block each (attention commutes with token permutation, so split blocks are
fed to the second core rotated by half).
"""

import numpy as np
import ml_dtypes
from contextlib import ExitStack

import concourse.bass as bass
import concourse.tile as tile
from concourse import bacc, mybir
from concourse import bass_utils

BLK = 48
C = 31
N = BLK * BLK  # 2304
NCORES = 8
SLOTS = 5
NT = N // 128  # 18 n-tiles
SCALE = 1.0 / float(np.sqrt(C))
CTX_SCALE = BLK / (BLK + C - 1.0)  # 48/78

GROUPS = [(0, 512), (512, 512), (1024, 512), (1536, 512), (2048, 256)]
# last slot of each core processes only the first half of a (possibly
# token-rotated) block: 36 blocks = 8 cores x (4 full + 1 half)
GROUPS_HALF = [(0, 512), (512, 512), (1024, 128)]
HALF = N // 2  # 1152
AR = 2  # a-tiles per score-matmul round (PSUM budget bound)
BF16 = mybir.dt.bfloat16
F32 = mybir.dt.float32

_BUILT = {}


def _build(slots=SLOTS, n_cores=NCORES, repeat=1, mirrors=True, pack=True,
           order=0, bufs_plus=False, diag=False, ar=AR):
    key = (slots, n_cores, repeat, mirrors, pack, order, bufs_plus,
           diag, ar)
    if key in _BUILT:
        return _BUILT[key]
    nc = bacc.Bacc("TRN2", target_bir_lowering=False, debug=False,
                   num_devices=n_cores)
    xb = nc.dram_tensor("xb", [slots, C, N], BF16, kind="ExternalInput").ap()
    w3t = nc.dram_tensor("w3t", [C, C], BF16, kind="ExternalInput").ap()
    out = nc.dram_tensor("out", [slots, C + 1, N], F32,
                         kind="ExternalOutput").ap()

    with tile.TileContext(nc) as tc, ExitStack() as ctx:
        bp = 1 if bufs_plus else 0
        const_pool = ctx.enter_context(tc.tile_pool(name="const", bufs=1))
        xpool = ctx.enter_context(tc.tile_pool(name="xpool", bufs=2 + bp))
        pk_pool = ctx.enter_context(tc.tile_pool(name="pk", bufs=1))
        xg_pool = ctx.enter_context(tc.tile_pool(name="xg", bufs=2 + bp))
        ctx_sb_pool = ctx.enter_context(
            tc.tile_pool(name="ctxsb", bufs=2 + bp))
        strip_pool = ctx.enter_context(
            tc.tile_pool(name="strips", bufs=2 + 2 * bp))
        psum_s = ctx.enter_context(
            tc.tile_pool(name="psum_s", bufs=2, space="PSUM"))
        pb = 2 if ar == 2 else 1
        psum_ctx = ctx.enter_context(
            tc.tile_pool(name="psum_ctx", bufs=pb, space="PSUM"))
        # xg staging and transpose staging share slots (disjoint in time)
        psum_t = ctx.enter_context(
            tc.tile_pool(name="psum_t", bufs=pb, space="PSUM"))
        psum_xg = psum_t

        w3t_sb = const_pool.tile([C, C], BF16)
        nc.sync.dma_start(w3t_sb[:], w3t[:])
        ident = const_pool.tile([128, 128], BF16)
        from concourse.masks import make_identity
        make_identity(nc, ident[:])

        half_last = (slots == SLOTS)
        for s in [s for _ in range(repeat) for s in range(slots)]:
            groups = GROUPS_HALF if (half_last and s == slots - 1) else GROUPS
            # x, replicated into the four 32-partition quadrants
            x4 = xpool.tile([128, N], BF16)
            for r in range(3):  # AP base partition must be in {0, 32, 64}
                nc.sync.dma_start(x4[32 * r:32 * r + C, :], xb[s])

            # xgT1[n, a, :31] = (w3 @ x)^T tile a; [..., 31] = 1.0
            xgT1 = xg_pool.tile([128, NT, 32], BF16)
            nc.vector.memset(xgT1[:], 1.0)
            for a0 in range(0, NT, 4):
                cnt = min(4, NT - a0)
                pxg = psum_xg.tile([128, 4, C], F32, tag="pt")
                for t in range(cnt):
                    a = a0 + t
                    nc.tensor.matmul(
                        pxg[:, t, :],
                        lhsT=x4[0:C, a * 128:(a + 1) * 128],
                        rhs=w3t_sb[:],
                        start=True, stop=True)
                nc.vector.tensor_copy(xgT1[:, a0:a0 + cnt, 0:C],
                                      pxg[:, 0:cnt, :])

            pk = pk_pool.tile([128, NT, N], BF16)  # P^T, [n-in-tile, a, m]
            ctxT = ctx_sb_pool.tile([C + 1, N], F32)

            def emit_st(gi):
                """Score matmuls + exp for group gi (direct tiles only).

                Rows inside the group's diagonal block are trimmed to the
                lower triangle (the rest arrives via emit_mirrors_diag)."""
                g0, W = groups[gi]
                u0 = g0 // 128
                a_start = u0 if mirrors else 0
                for a0 in range(a_start, NT, ar):
                    cnt = min(ar, NT - a0)
                    ps = psum_s.tile([128, ar, 512], F32, name="ps")
                    widths = []
                    for t in range(cnt):
                        a = a0 + t
                        Wa = W
                        if mirrors and diag:
                            Wa = min(W, (a - u0 + 1) * 128)
                        widths.append(Wa)
                        r = (a % 3) if pack else 0
                        nc.tensor.matmul(
                            ps[:, t, 0:Wa],
                            lhsT=x4[32 * r:32 * r + C,
                                    a * 128:(a + 1) * 128],
                            rhs=x4[32 * r:32 * r + C, g0:g0 + Wa],
                            start=True, stop=True)
                    if len(set(widths)) == 1:
                        nc.scalar.activation(
                            pk[:, a0:a0 + cnt, g0:g0 + widths[0]],
                            ps[:, 0:cnt, 0:widths[0]],
                            mybir.ActivationFunctionType.Exp, scale=SCALE)
                    else:
                        for t in range(cnt):
                            nc.scalar.activation(
                                pk[:, a0 + t, g0:g0 + widths[t]],
                                ps[:, t, 0:widths[t]],
                                mybir.ActivationFunctionType.Exp, scale=SCALE)

            def emit_mirrors(gi):
                # Upper-triangle tiles of P^T by symmetry: PE-transpose the
                # exp'd lower-triangle mirror tile (bf16 via PSUM), evacuate
                # four tiles per DVE copy (2x bf16 mode).
                g0, W = groups[gi]
                k = g0 // 128
                for b in range(g0 // 128, (g0 + W) // 128):
                    for a0 in range(0, k, 4):
                        cnt = min(4, k - a0)
                        pt = psum_t.tile([128, 4, 128], BF16, name="pt")
                        for t in range(cnt):
                            a = a0 + t
                            nc.tensor.transpose(
                                pt[:, t, :],
                                pk[:, b, a * 128:(a + 1) * 128],
                                ident[:])
                        nc.vector.tensor_copy(
                            pk[:, a0:a0 + cnt, b * 128:(b + 1) * 128],
                            pt[:, 0:cnt, :])

            def emit_pv(gi):
                g0, W = groups[gi]
                pctx = psum_ctx.tile([128, 512], F32, name="pctx")
                for a in range(NT):
                    t = (a % 3) if pack else 0  # PE column group / PSUM strip
                    nc.tensor.matmul(
                        pctx[32 * t:32 * t + 32, 0:W],
                        lhsT=xgT1[:, a, :],
                        rhs=pk[:, a, g0:g0 + W],
                        start=(a < 3), stop=(a >= NT - 3),
                        skip_group_check=True)
                if pack:
                    # DVE can read only one PSUM operand per instruction
                    t0 = strip_pool.tile([32, 512], F32, name="t0")
                    nc.vector.tensor_copy(t0[:, 0:W], pctx[0:32, 0:W])
                    t01 = strip_pool.tile([32, 512], F32, name="t01")
                    nc.vector.tensor_add(t01[:, 0:W], t0[:, 0:W],
                                         pctx[32:64, 0:W])
                    nc.vector.tensor_add(ctxT[:, g0:g0 + W], t01[:, 0:W],
                                         pctx[64:96, 0:W])
                else:
                    nc.vector.tensor_copy(ctxT[:, g0:g0 + W], pctx[0:32, 0:W])

            def emit_mirrors_diag(gi):
                # Within-group upper-triangle tiles (a < b, both in group):
                # sources are this group's freshly exp'd lower rows.
                g0, W = groups[gi]
                u0 = g0 // 128
                nb = W // 128
                for b in range(u0 + 1, u0 + nb):
                    k = b - u0
                    pt = psum_t.tile([128, 4, 128], BF16, name="pt")
                    for t in range(k):
                        a = u0 + t
                        nc.tensor.transpose(
                            pt[:, t, :],
                            pk[:, b, a * 128:(a + 1) * 128],
                            ident[:])
                    nc.vector.tensor_copy(
                        pk[:, u0:u0 + k, b * 128:(b + 1) * 128],
                        pt[:, 0:k, :])

            # software pipeline at group level: ST(g+1) overlaps PV(g)
            for gi in range(len(groups)):
                if order == 0:
                    if mirrors:
                        emit_mirrors(gi)
                    emit_st(gi)
                    if mirrors and diag:
                        emit_mirrors_diag(gi)
                    if gi > 0:
                        emit_pv(gi - 1)
                elif order == 1:
                    emit_st(gi)
                    if mirrors:
                        emit_mirrors(gi)
                    if gi > 0:
                        emit_pv(gi - 1)
                else:
                    emit_st(gi)
                    if gi > 0:
                        emit_pv(gi - 1)
                    if mirrors:
                        emit_mirrors(gi)
            emit_pv(len(groups) - 1)

            mtot = sum(w for _, w in groups)
            nc.sync.dma_start(out[s][:, 0:mtot], ctxT[:, 0:mtot])

    nc.compile()
    _BUILT[key] = nc
    return nc


def make_in_maps(xb_all, w3):
    """Per-core input maps: 4 full blocks + 1 half block per core.

    Blocks 0..31 -> core c slots 0..3.  Blocks 32+k are split: core 2k's
    slot 4 sees the block as-is (computes tokens [0,1152)); core 2k+1's
    slot 4 sees it rotated by 1152 tokens (computes the other half --
    attention commutes with any joint token permutation)."""
    xb_bf16 = xb_all.astype(ml_dtypes.bfloat16)
    w3t = np.ascontiguousarray(w3[:, :, 0, 0].T).astype(ml_dtypes.bfloat16)
    in_maps = []
    for ci_ in range(NCORES):
        sl = [xb_bf16[4 * ci_ + s] for s in range(4)]
        xh = xb_bf16[32 + ci_ // 2]
        if ci_ % 2 == 1:
            xh = np.concatenate([xh[:, HALF:], xh[:, :HALF]], axis=1)
        sl.append(xh)
        in_maps.append({"xb": np.ascontiguousarray(np.stack(sl)),
                        "w3t": w3t})
    return in_maps


def _conv2d_np(x, w, pad):
    """x: (ci, h, w) f32; w: (co, ci, kh, kw); zero padding `pad`."""
    ci, h, wd = x.shape
    co, _, kh, kw = w.shape
    xp = np.zeros((ci, h + 2 * pad, wd + 2 * pad), np.float32)
    xp[:, pad:pad + h, pad:pad + wd] = x
    out = np.zeros((co, h, wd), np.float32)
    for dy in range(kh):
        for dx in range(kw):
            out += np.einsum('oi,iyx->oyx', w[:, :, dy, dx],
                             xp[:, dy:dy + h, dx:dx + wd],
                             optimize=True)
    return out


def kernel(xt, w1, w2, w3, wz1):
    xt = np.asarray(xt)
    w1 = np.asarray(w1)
    w2 = np.asarray(w2)
    w3 = np.asarray(w3)
    wz1 = np.asarray(wz1)

    b, c, h_inp, w_inp = xt.shape
    assert (b, c, h_inp, w_inp) == (1, C, 256, 256)
    pad_h = (-h_inp) % BLK
    pad_w = (-w_inp) % BLK
    xpad = np.pad(xt, ((0, 0), (0, 0), (0, pad_h), (0, pad_w)), mode='reflect')
    H, W = h_inp + pad_h, w_inp + pad_w
    nh, nw = H // BLK, W // BLK
    nblk = nh * nw  # 36
    xb_all = xpad.reshape(c, nh, BLK, nw, BLK).transpose(1, 3, 0, 2, 4)
    xb_all = np.ascontiguousarray(xb_all).reshape(nblk, c, N)

    in_maps = make_in_maps(xb_all, w3)

    nc = _build()
    res = bass_utils.run_bass_kernel_spmd(
        nc, in_maps, core_ids=list(range(NCORES)))

    # Host-side epilogue: normalize, scale, relu, relayout.
    ctxm_all = {}
    half_parts = {}
    for ci_ in range(NCORES):
        o = res.results[ci_]["out"]  # [SLOTS, 32, N] f32
        for s in range(4):
            ctxT = o[s]
            ctxm_all[4 * ci_ + s] = ctxT[:C] * (CTX_SCALE / ctxT[C])
        ctxTh = o[4][:, :HALF]
        half_parts.setdefault(32 + ci_ // 2, {})[ci_ % 2] = (
            ctxTh[:C] * (CTX_SCALE / ctxTh[C]))
    for blk, parts in half_parts.items():
        ctxm_all[blk] = np.concatenate([parts[0], parts[1]], axis=1)

    zpad = np.zeros((c, H, W), np.float32)
    use_conv = float(wz1.reshape(-1)[0]) != 0.0
    wz = np.float32(wz1.reshape(-1)[0])
    for blk in range(nblk):
        zblk = np.maximum(ctxm_all[blk].T, 0.0).reshape(c, BLK, BLK)
        if use_conv:
            xblk = xb_all[blk].reshape(c, BLK, BLK).astype(np.float32)
            h1 = np.maximum(_conv2d_np(xblk, w1, 1), 0.0)
            z1 = xblk + _conv2d_np(h1, w2, 1)
            zblk = wz * z1 + (1.0 - wz) * zblk
        i, j = blk // nw, blk % nw
        zpad[:, i * BLK:(i + 1) * BLK, j * BLK:(j + 1) * BLK] = zblk

    return zpad[None, :, :h_inp, :w_inp].astype(np.float32)
